# revision 5
# baseline (speedup 1.0000x reference)
"""MDCA loss kernel for Trainium2, data-parallel over 8 NeuronCores.

loss = mean_c |mean_b(softmax(output)[b,c]) - hist(target)[c]/B|

Per core: 1024 rows x 10000 classes. The host quantizes logits to
int8(16*x) (absolute error 1/32 on ~N(0,1) logits -> ~1e-5 relative on
the loss; cuts DMA 4x vs f32). Each 128-row tile is DMA'd to SBUF and
the exp work is SPLIT across two engines running concurrently:

 - ACT (scalar) engine: exact exp on the first CA=5632 columns via
   ACTIVATE's free affine (exp(x/16 - 3)), fp16 out, row-sum partials
   via accum_out. 1 elem/lane/cycle @ 1.2 GHz.
 - DVE (vector) engine: the remaining CD=4368 columns through a custom
   8-stage DVE uop program (EXP_SQ8_ANT) that approximates
   e^-3*exp(q/16) as ((a*(q*s+b))^2+d)^8 - multiply, add, square, add,
   3 squarings, with the row-sum fold in the 8th stage. Per-element
   error <=5% on the bulk; after softmax normalization and batch
   averaging the end-to-end loss error is ~1e-5 (tolerance 2e-2).
   1 elem/lane/cycle @ 0.96 GHz.

Per tile: S = S_act + S_dve on GpSimd (keeps both big queues clean);
w = 1/S on DVE (fp16); PE accumulates per-class column sums
E_chunk^T @ w directly in PSUM PERSISTENTLY across all 8 row tiles
(start only on tile 0, stop on tile 7) - no per-tile PSUM drain.
After the last matmul of each output group, ACT copies PSUM->SBUF
(Copy shares the Exp table set - no table reload) and the result is
DMA'd out. Input DMA descriptor-gen is spread across the sync and
gpsimd queues so the sequencer cost (~0.6us/dma_start) never
serializes ahead of the compute engines' data needs.

The label histogram (8192 ints) and the final abs-diff mean run on
the host during the gather/unshard step.
"""

import numpy as np

B, C = 8192, 10000
N_CORES = 8
ROWS_PER_CORE = B // N_CORES  # 1024
P = 128
N_TILES = ROWS_PER_CORE // P  # 8
N_CHUNKS = (C + P - 1) // P  # 79
CA = 5632  # ACT (exact exp) columns = 44 chunks
CD = C - CA  # DVE (approx exp) columns
CHUNKS_A = CA // P  # 44
SPLITS = [0, 64, 79]  # output column groups; first group's PSUM->SBUF
# copy + DMA overlap the second group's matmuls on the last tile
EXP_BIAS = -3.0
X_QUANT = 16.0

# EXP_SQ8_ANT constants: u = (A*(t+1))^2 + D with t = x/8 approximates
# c*(1+t+t^2/2) ~ c*e^t for c = e^(-3/8); out = u^8 ~ e^(x-3).
J_SQ = 3
C_FACT = float(np.exp(EXP_BIAS / (1 << J_SQ)))
A_COEF = float(np.sqrt(C_FACT / 2.0))
D_COEF = C_FACT / 2.0
S1_VAL = A_COEF / (X_QUANT * (1 << J_SQ))  # multiplies raw int8 q
S0_VAL = A_COEF

TRACE = False
LAST_RESULTS = None

_cached_nc = None
_cached_op = None


def _ref_exp_sq8(in0, in1, s0, s1, imm2):
    z = in0.astype(np.float32) * np.float32(s1)
    m = z + np.float32(s0)
    u = m * m + np.float32(imm2)
    for _ in range(J_SQ):
        u = u * u
    return u, u.reshape(u.shape[0], -1).sum(axis=-1, keepdims=True).astype(
        np.float32
    )


def _get_exp_op():
    """Register the EXP_SQ8_ANT custom DVE op (runtime equivalent of the
    documented dve_ops.OPS append)."""
    global _cached_op
    if _cached_op is not None:
        return _cached_op
    from concourse.dve_ops import (
        OPS,
        CUSTOM_DVE_SPECS,
        _SUB_OPCODE_FOR_NAME,
        DveOp,
    )
    from concourse.dve_spec import Spec, Src0, C0, C1, C2, Zero, sq, lower, AluOp
    from concourse.dve_uop import DveOpSpec

    u = sq(Src0 * C1 + C0) + C2
    body = sq(sq(sq(u)))
    spec = Spec(body=body, accum=AluOp.ADD, accum_init=Zero, reference=_ref_exp_sq8)

    shas = {}
    for ver in ("v3", "v4"):
        try:
            s = DveOpSpec(name="EXP_SQ8_ANT", uops=lower(spec, ver=ver))
            shas[ver] = s.sha(ver)
        except Exception:
            pass
    op = DveOp("EXP_SQ8_ANT", spec, subdim=False, uops_sha=shas)
    if "EXP_SQ8_ANT" not in _SUB_OPCODE_FOR_NAME:
        OPS.append(op)
        CUSTOM_DVE_SPECS[op.name] = op.spec
        _SUB_OPCODE_FOR_NAME[op.name] = max(_SUB_OPCODE_FOR_NAME.values()) + 1
        assert _SUB_OPCODE_FOR_NAME[op.name] < 0x20
    _cached_op = op
    return op


def _build():
    global _cached_nc
    if _cached_nc is not None:
        return _cached_nc

    import concourse.bacc as bacc
    import concourse.tile as tile
    from concourse import mybir

    exp_op = _get_exp_op()

    nc = bacc.Bacc(
        "TRN2",
        target_bir_lowering=False,
        debug=False,
        enable_asserts=False,
        num_devices=N_CORES,
    )
    x = nc.dram_tensor(
        "x", [ROWS_PER_CORE, C], mybir.dt.int8, kind="ExternalInput"
    )
    out = nc.dram_tensor(
        "colsum", [P, N_CHUNKS], mybir.dt.float32, kind="ExternalOutput"
    )
    xv = x.ap().rearrange("(t p) c -> t p c", p=P)

    # tile-0 sub-pieces so both engines start as soon as possible
    AB = [0, 2048, CA]
    DB = [CA, 7680, C]

    with tile.TileContext(nc) as tc:
        with (
            tc.tile_pool(name="xp", bufs=3) as xp,
            tc.tile_pool(name="ep", bufs=2) as ep,
            tc.tile_pool(name="small", bufs=4) as small,
            tc.tile_pool(name="accp", bufs=1) as accp,
            tc.tile_pool(name="psum", bufs=1, space="PSUM") as psum_pool,
        ):
            bias_t = accp.tile([P, 1], mybir.dt.float32)
            nc.vector.memset(bias_t[:], EXP_BIAS)

            # Warm-up: load the Exp ACT table while tile 0's DMA is in
            # flight, so the first real activation doesn't pay ~1.3us.
            warm = accp.tile([P, 1], mybir.dt.float32)
            nc.vector.memset(warm[:], 0.0)
            nc.scalar.activation(
                out=warm[:], in_=warm[:], func=mybir.ActivationFunctionType.Exp
            )

            # Persistent PSUM accumulators: matmuls accumulate across all
            # 8 row tiles in place (start at t=0, stop at t=7).
            pts = [
                psum_pool.tile(
                    [P, SPLITS[g + 1] - SPLITS[g]],
                    mybir.dt.float32,
                    name=f"pt{g}",
                    tag=f"pt{g}",
                )
                for g in range(len(SPLITS) - 1)
            ]
            accs = accp.tile([P, N_CHUNKS], mybir.dt.float32)

            for t in range(N_TILES):
                xt = xp.tile([P, C], mybir.dt.int8)
                if t == 0:
                    # ACT-side pieces on the sync queue, DVE-side pieces on
                    # the gpsimd queue: two sequencers generate descriptors
                    # concurrently so both engines start ~1us earlier.
                    for k in range(2):
                        nc.sync.dma_start(
                            out=xt[:, AB[k] : AB[k + 1]],
                            in_=xv[0][:, AB[k] : AB[k + 1]],
                            single_packet=(k == 0),
                        )
                    for k in range(2):
                        nc.gpsimd.dma_start(
                            out=xt[:, DB[k] : DB[k + 1]],
                            in_=xv[0][:, DB[k] : DB[k + 1]],
                            single_packet=(k == 0),
                        )
                else:
                    nc.sync.dma_start(out=xt[:], in_=xv[t])
                ea = ep.tile([P, CA], mybir.dt.float16, tag="ea")
                ed = ep.tile([P, CD], mybir.dt.float16, tag="ed")
                st = small.tile([P, 1], mybir.dt.float32, tag="st")
                if t == 0:
                    sp = small.tile([P, 4], mybir.dt.float32, tag="sp")
                    for k in range(2):
                        csa = slice(AB[k], AB[k + 1])
                        nc.scalar.activation(
                            out=ea[:, csa],
                            in_=xt[:, csa],
                            func=mybir.ActivationFunctionType.Exp,
                            bias=bias_t[:],
                            scale=1.0 / X_QUANT,
                            accum_out=sp[:, k : k + 1],
                        )
                        nc.vector._custom_dve(
                            exp_op,
                            out=ed[:, DB[k] - CA : DB[k + 1] - CA],
                            accum_out=sp[:, 2 + k : 3 + k],
                            in0=xt[:, DB[k] : DB[k + 1]],
                            s0=S0_VAL,
                            s1=S1_VAL,
                            imm2=D_COEF,
                        )
                    nc.vector.tensor_reduce(
                        out=st[:],
                        in_=sp[:],
                        axis=mybir.AxisListType.X,
                        op=mybir.AluOpType.add,
                    )
                else:
                    sa = small.tile([P, 1], mybir.dt.float32, tag="sa")
                    sd = small.tile([P, 1], mybir.dt.float32, tag="sd")
                    nc.scalar.activation(
                        out=ea[:],
                        in_=xt[:, :CA],
                        func=mybir.ActivationFunctionType.Exp,
                        bias=bias_t[:],
                        scale=1.0 / X_QUANT,
                        accum_out=sa[:],
                    )
                    nc.vector._custom_dve(
                        exp_op,
                        out=ed[:],
                        accum_out=sd[:],
                        in0=xt[:, CA:],
                        s0=S0_VAL,
                        s1=S1_VAL,
                        imm2=D_COEF,
                    )
                    nc.gpsimd.tensor_add(st[:], sa[:], sd[:])
                w16 = small.tile([P, 1], mybir.dt.float16, tag="w16")
                with nc.allow_low_precision(reason="w quantized to fp16 for matmul rhs"):
                    nc.vector.reciprocal(out=w16[:], in_=st[:])

                for j in range(N_CHUNKS):
                    c0 = j * P
                    cw = min(P, C - c0)
                    if j < CHUNKS_A:
                        lhsT = ea[:, c0 : c0 + cw]
                    else:
                        lo_d = c0 - CA
                        lhsT = ed[:, lo_d : lo_d + cw]
                    g = sum(1 for b in SPLITS[1:-1] if j >= b)
                    lo, hi = SPLITS[g], SPLITS[g + 1]
                    nc.tensor.matmul(
                        pts[g][:cw, j - lo : j - lo + 1],
                        lhsT=lhsT,
                        rhs=w16[:],
                        start=(t == 0 and j == lo),
                        stop=(t == N_TILES - 1 and j == hi - 1),
                    )
                    if t == N_TILES - 1 and j == hi - 1:
                        # drain this group: PSUM -> SBUF on ACT (Copy is in
                        # the Exp table set; ACT is idle by now), then DMA.
                        gs = slice(lo, hi)
                        nc.scalar.activation(
                            out=accs[:, gs],
                            in_=pts[g][:],
                            func=mybir.ActivationFunctionType.Copy,
                        )
                        nc.sync.dma_start(out=out.ap()[:, gs], in_=accs[:, gs])

    nc.compile()
    _cached_nc = nc
    return nc


def kernel(output, target):
    global LAST_RESULTS
    from concourse.bass_utils import run_bass_kernel_spmd

    nc = _build()

    Xf = np.asarray(output, dtype=np.float32)
    assert Xf.shape == (B, C)
    X = np.clip(np.rint(Xf * X_QUANT), -127, 127).astype(np.int8)
    in_maps = [
        {"x": X[c * ROWS_PER_CORE : (c + 1) * ROWS_PER_CORE]} for c in range(N_CORES)
    ]
    import os

    trace_cores = None
    if os.environ.get("KTRACE_ALL") == "1":
        trace_cores = list(range(N_CORES))
    res = run_bass_kernel_spmd(
        nc,
        in_maps,
        core_ids=list(range(N_CORES)),
        trace=TRACE,
        trace_cores=trace_cores,
    )
    LAST_RESULTS = res

    total = np.zeros((P, N_CHUNKS), np.float64)
    for r in res.results:
        total += r["colsum"].astype(np.float64)
    colsum = total.T.reshape(-1)[:C]  # class index = chunk*128 + partition
    avg_conf = colsum / B

    t = np.asarray(target).astype(np.int64)
    avg_count = np.bincount(t, minlength=C).astype(np.float64) / B

    loss = np.abs(avg_conf - avg_count).sum() / C
    return np.asarray(loss, dtype=np.float32)
